# revision 12
# baseline (speedup 1.0000x reference)
"""Trainium2 Bass kernel: LADIES mini-batch ER-GCN (2-layer relational GCN).

Contract: kernel(**inputs) takes the FULL unsharded inputs (numpy, keyed as in
setup_inputs) and returns the FULL [256, 32] float32 output.

Strategy (8 NeuronCores, relation-sharded layer 1, output-row-sharded layer 2):
  - h1 = relu(A0 @ xw + b1) dominates: A0 is [1024, 131072] f32 = 512 MB.
    Core c owns relations {2c, 2c+1} = a contiguous 64 MB column block of A0.
    The block is transposed AND pre-swizzled into exact DMA order on the host,
    then streamed as fp8 e3m4 (scale 512 folded into w1; quantization error
    ~1.6e-2 rel-absmax, under the 2e-2 gate) -> 16.8 MB/core, contiguous
    per-partition runs, ~1.3 MB per dma_start on the sync HWDGE ring.
  - Each core computes xw[r] = x @ w1[r] for its 2 relations on-device (bf16),
    then accumulates h1.T = sum_k xw[k-tile].T @ A0T[k-tile] in PSUM, in 2
    asymmetric column chunks (640 then 384) so the first chunk's AllReduce
    hides under the second chunk's streaming and the exposed final AllReduce
    is small (48 KB bf16).
  - A tiny warmup AllReduce fires ~3 us into the kernel: it absorbs the ncfw
    first-collective warmup (~20-30 us) and synchronizes the 8 ranks so the
    real AllReduces start promptly when triggered.
  - Layer 2: after the AllReduce every core has full h1, so core c computes
    out.T[:, 32c:32c+32] (its 32 output rows) against a host-packed A1.T
    column block -- no second collective; the host concatenates the slices.
    The M=32 out-matmuls run 3-at-a-time in distinct PE column groups.
"""

import numpy as np
import ml_dtypes

# Problem dimensions (fixed by the problem spec).
R, NB = 16, 16
N2, N1, NOUT = 8192, 1024, 256
F, E, C = 128, 64, 32

NCORES = 8
RPC = R // NCORES            # relations per core = 2
KPC = RPC * N2               # layer-1 contraction rows per core = 16384
NKT = KPC // 128             # k-tiles per core = 128
NB2 = N2 // 128              # n2-blocks per relation = 64
CHUNKS = [640, 384]          # n1 column chunks (AllReduce pipelining)
KT_PER_DMA = 16              # k-tiles per A0 DMA (~1.3 MB fp8 transfers)
NOPC = NOUT // NCORES        # output rows per core = 32
XWB = 4                      # xw n2-blocks batched per PSUM bank/cast

A0_SCALE = 512.0             # power-of-2 scale folded into w1 (fp8 range fit)

_cache = {}
last_results = None          # BassKernelResults from the most recent run


def _subtiles(width):
    """Split a chunk width into PSUM-bank-sized (<=512 f32) sub-widths."""
    subs = []
    off = 0
    while off < width:
        w = min(512, width - off)
        subs.append((off, w))
        off += w
    return subs


def _build_module(repeats=1, use_collectives=True):
    import concourse.bacc as bacc
    import concourse.tile as tile
    import concourse.tile as tile_mod
    import concourse.mybir as mybir

    f32 = mybir.dt.float32
    bf16 = mybir.dt.bfloat16
    f8 = mybir.dt.float8e3

    nc = bacc.Bacc("TRN2", target_bir_lowering=False, debug=False,
                   num_devices=NCORES)

    xt = nc.dram_tensor("xt", [F, N2], bf16, kind="ExternalInput")
    a0t = nc.dram_tensor("a0t", [128, NKT * N1], f8, kind="ExternalInput")
    a1t = nc.dram_tensor("a1t", [128, NKT * C], bf16, kind="ExternalInput")
    w1c = nc.dram_tensor("w1c", [F, RPC * E], bf16, kind="ExternalInput")
    w2a = nc.dram_tensor("w2a", [E, R * C], f32, kind="ExternalInput")
    b1 = nc.dram_tensor("b1", [E, 1], f32, kind="ExternalInput")
    b2 = nc.dram_tensor("b2", [C, 1], f32, kind="ExternalInput")
    outT = nc.dram_tensor("outT", [C, NOPC], f32, kind="ExternalOutput")

    # host pre-swizzled: per chunk ch (column range), [p, t, n'] contiguous
    chunk_off = [0]
    for w in CHUNKS:
        chunk_off.append(chunk_off[-1] + w)
    rg = [list(range(NCORES))]

    with tile.TileContext(nc) as tc:
        with (
            tc.tile_pool(name="const", bufs=1) as constp,
            tc.tile_pool(name="xtp", bufs=1) as xtp,
            tc.tile_pool(name="xwp", bufs=1) as xwp,
            tc.tile_pool(name="a0p", bufs=8) as a0p,
            tc.tile_pool(name="a1p", bufs=1) as a1p,
            tc.tile_pool(name="h1p", bufs=2) as h1p,
            tc.tile_pool(name="h2p", bufs=8) as h2p,
            tc.tile_pool(name="warm", bufs=1) as warmp,
            tc.tile_pool(name="psxw", bufs=2, space="PSUM") as psxw,
            tc.tile_pool(name="psh1", bufs=1, space="PSUM") as psh1,
            tc.tile_pool(name="psh2", bufs=2, space="PSUM") as psh2,
            tc.tile_pool(name="psout", bufs=1, space="PSUM") as psoutp,
            tc.tile_pool(name="dram", bufs=1, space="DRAM") as dramp,
        ):
            # ---- warmup AllReduce: absorbs ncfw first-op cost + rank sync --
            warm_sb = warmp.tile([1, 32], f32, name="warm_sb")
            nc.vector.memset(warm_sb[:], 0.0)
            warm_in = dramp.tile([1, 32], f32, name="warm_in")
            warm_out = dramp.tile([1, 32], f32, name="warm_out",
                                  addr_space="Shared")
            nc.gpsimd.dma_start(warm_in[:], warm_sb[:])
            if use_collectives:
                nc.gpsimd.collective_compute(
                    "AllReduce",
                    mybir.AluOpType.add,
                    replica_groups=rg,
                    ins=[warm_in.opt()],
                    outs=[warm_out.opt()],
                )

            # ---- input loads ----
            # xt first half on the (otherwise idle early) gpsimd SWDGE ring,
            # second half on the scalar HWDGE ring: the xw matmuls can start
            # ~8 us earlier than a single 2 MB load behind the scalar
            # preamble. A0 owns the sync ring; a1 is queued on it between the
            # two chunks (needed only in the post phase).
            xt_sb = [xtp.tile([F, N2 // 2], bf16, name=f"xt_sb{h}")
                     for h in range(2)]
            nc.gpsimd.dma_start(xt_sb[0][:], xt[:, 0:N2 // 2])
            nc.scalar.dma_start(xt_sb[1][:], xt[:, N2 // 2:N2])
            w1_sb = constp.tile([F, RPC * E], bf16, name="w1_sb")
            nc.scalar.dma_start(w1_sb[:], w1c[:])
            w2_sb = constp.tile([E, R * C], f32, name="w2_sb")
            nc.scalar.dma_start(w2_sb[:], w2a[:])
            b1_sb = constp.tile([E, 1], f32, name="b1_sb")
            nc.scalar.dma_start(b1_sb[:], b1[:])
            b2_sb = constp.tile([C, 1], f32, name="b2_sb")
            nc.scalar.dma_start(b2_sb[:], b2[:])
            a1_sb = a1p.tile([128, NKT * C], bf16, name="a1_sb")

            for rep in range(repeats):
                # ---- xw[kt] = x[n2-block] @ w1[r_local] (kt = rl*64+nb) --
                # one matmul per n2-block computes BOTH relations (N=128).
                # XWB blocks share one PSUM bank so a single wide strided
                # cast drains them -- keeps PE matmuls back-to-back (HAM
                # warm) instead of serializing on per-block DVE casts.
                xw_sb = xwp.tile([128, NKT * E], bf16, name="xw_sb",
                                 tag="xw_sb")
                xw_v = xw_sb[:].rearrange("p (rl nb e) -> p nb rl e",
                                          rl=RPC, e=E)
                for g in range(NB2 // XWB):
                    ps = psxw.tile([128, XWB * RPC * E], f32, name="ps_xw",
                                   tag="ps_xw")
                    for q in range(XWB):
                        nb = g * XWB + q
                        xth = xt_sb[nb // (NB2 // 2)]
                        nbl = nb % (NB2 // 2)
                        nc.tensor.matmul(
                            ps[:, q * RPC * E:(q + 1) * RPC * E],
                            xth[:, nbl * 128:(nbl + 1) * 128],
                            w1_sb[:],
                            start=True, stop=True,
                        )
                    nc.vector.tensor_copy(
                        xw_v[:, g * XWB:(g + 1) * XWB],
                        ps[:].rearrange("p (nb rl e) -> p nb rl e",
                                        nb=XWB, e=E))

                # collective bounce buffers (one pair per chunk, bf16)
                cc_in = [dramp.tile([E, w], bf16, name=f"cc_in{rep}_{ch}")
                         for ch, w in enumerate(CHUNKS)]
                cc_out = [dramp.tile([E, w], bf16, name=f"cc_out{rep}_{ch}",
                                     addr_space="Shared")
                          for ch, w in enumerate(CHUNKS)]

                # ---- stream phase: all chunks' A0 matmuls, then their ARs --
                stream_mms = [[] for _ in CHUNKS]
                for ch, cw in enumerate(CHUNKS):
                    a0_view = a0t.ap()[:, chunk_off[ch] * NKT:
                                       chunk_off[ch] * NKT + NKT * cw]
                    a0_view = a0_view.rearrange("p (t n) -> p t n", n=cw)
                    subs = _subtiles(cw)
                    ps_h1 = [psh1.tile([E, sw], f32, name=f"ps_h1_{ch}_{si}",
                                       tag=f"ps_h1_{ch}_{si}")
                             for si, (soff, sw) in enumerate(subs)]
                    for g in range(NKT // KT_PER_DMA):
                        a0_sb = a0p.tile([128, KT_PER_DMA, cw], f8,
                                         name="a0_sb", tag="a0")
                        nc.sync.dma_start(
                            a0_sb[:],
                            a0_view[:, g * KT_PER_DMA:(g + 1) * KT_PER_DMA,
                                    :],
                        )
                        for i in range(KT_PER_DMA):
                            kt = g * KT_PER_DMA + i
                            for si, (soff, sw) in enumerate(subs):
                                mm = nc.tensor.matmul(
                                    ps_h1[si][:],
                                    xw_sb[:, kt * E:(kt + 1) * E],
                                    a0_sb[:, i, soff:soff + sw],
                                    start=(kt == 0), stop=(kt == NKT - 1),
                                )
                                stream_mms[ch].append(mm)
                    if ch == 0:
                        # a1 rides the sync ring between the two chunks
                        nc.sync.dma_start(a1_sb[:], a1t[:])
                    h1part = h1p.tile([E, cw], bf16, name=f"h1part{ch}",
                                      tag=f"h1part{ch}")
                    for si, (soff, sw) in enumerate(subs):
                        nc.vector.tensor_copy(h1part[:, soff:soff + sw],
                                              ps_h1[si][:])
                    nc.scalar.dma_start(cc_in[ch][:], h1part[:])
                    if use_collectives:
                        nc.gpsimd.collective_compute(
                            "AllReduce",
                            mybir.AluOpType.add,
                            replica_groups=rg,
                            ins=[cc_in[ch].opt()],
                            outs=[cc_out[ch].opt()],
                        )
                    else:  # single-core timing variant
                        nc.gpsimd.dma_start(cc_out[ch][:], cc_in[ch][:])

                # ---- post phase: per chunk, relu + layer 2 + out-MMs ----
                # out-MMs are M=32: run 3 concurrently in distinct PE column
                # groups (partition offsets 0/32/64 of one PSUM bank, via
                # auto tile_position), one accumulation group per r%3;
                # partials summed on DVE at the end.
                ps_out = psoutp.tile([96, NOPC], f32, name="ps_out",
                                     tag="ps_out")
                nblk_total = N1 // 128
                nper = [0, 0, 0]
                for r in range(R):
                    nper[r % 3] += nblk_total
                igrp = [0, 0, 0]
                for ch, cw in enumerate(CHUNKS):
                    nb1c = cw // 128
                    h1s = h1p.tile([E, cw], bf16, name=f"h1s{ch}",
                                   tag=f"h1s{ch}")
                    nc.scalar.dma_start(h1s[:], cc_out[ch][:])
                    h1r = h1p.tile([E, cw], f32, name=f"h1r{ch}",
                                   tag=f"h1r{ch}")
                    nc.scalar.activation(
                        h1r[:], h1s[:],
                        mybir.ActivationFunctionType.Relu,
                        bias=b1_sb[:],
                    )

                    # h2 for ALL 16 relations of one n1-block in a single
                    # N=512 matmul (out columns are (r, c) pairs, r-major,
                    # matching w2a/a1t layout). Anchor this chunk's PE work
                    # behind ~75% of the NEXT chunk's stream matmuls so the
                    # scheduler can't hoist it ahead and head-of-line-block
                    # PE on the AllReduce.
                    if ch + 1 < len(CHUNKS):
                        nxt = stream_mms[ch + 1]
                        anchor = nxt[(3 * len(nxt)) // 4].ins
                    else:
                        anchor = None
                    h2ts = {}
                    for b in range(nb1c):
                        ps2 = psh2.tile([128, R * C], f32, name="ps_h2",
                                        tag="ps_h2")
                        mm = nc.tensor.matmul(
                            ps2[:],
                            h1r[:, b * 128:(b + 1) * 128],
                            w2_sb[:],
                            start=True, stop=True,
                        )
                        if anchor is not None:
                            tile_mod.add_dep_helper(
                                mm.ins, anchor, sync=False,
                                reason="post-phase after next stream")
                        h2t = h2p.tile([128, R * C], bf16, name="h2t",
                                       tag="h2t")
                        nc.vector.tensor_copy(h2t[:], ps2[:])
                        h2ts[b] = h2t

                    for b in range(nb1c):
                        nb1 = chunk_off[ch] // 128 + b
                        for r in range(R):
                            t = r * (N1 // 128) + nb1
                            q = r % 3
                            nc.tensor.matmul(
                                ps_out[32 * q:32 * (q + 1), :],
                                h2ts[b][:, r * C:(r + 1) * C],
                                a1_sb[:, t * C:(t + 1) * C],
                                start=(igrp[q] == 0),
                                stop=(igrp[q] == nper[q] - 1),
                                skip_group_check=True,
                            )
                            igrp[q] += 1

                # ---- sum the 3 col-group partials + bias2, store out.T ----
                t01 = constp.tile([C, NOPC], f32, name="t01", tag="t01")
                out_sb = constp.tile([C, NOPC], f32, name="out_sb",
                                     tag="out_sb")
                nc.vector.tensor_copy(t01[:], ps_out[0:32, :])
                nc.vector.tensor_add(t01[:], ps_out[32:64, :], t01[:])
                nc.vector.tensor_add(t01[:], ps_out[64:96, :], t01[:])
                nc.vector.tensor_scalar_add(out_sb[:], t01[:], b2_sb[:])
                nc.scalar.dma_start(outT[:], out_sb[:])


    nc.compile()
    return nc


def _get_module():
    if "nc" not in _cache:
        _cache["nc"] = _build_module()
    return _cache["nc"]


def make_in_maps(X_batch, sel_idx, A0, A1, comp1, bases1, comp2, bases2,
                 bias1, bias2):
    """Host-side sharding / layout prep -> per-core input maps."""
    X_batch = np.asarray(X_batch, dtype=np.float32)
    sel_idx = np.asarray(sel_idx)
    A0 = np.asarray(A0, dtype=np.float32)
    A1 = np.asarray(A1, dtype=np.float32)
    comp1 = np.asarray(comp1, dtype=np.float32)
    bases1 = np.asarray(bases1, dtype=np.float32)
    comp2 = np.asarray(comp2, dtype=np.float32)
    bases2 = np.asarray(bases2, dtype=np.float32)
    bias1 = np.asarray(bias1, dtype=np.float32)
    bias2 = np.asarray(bias2, dtype=np.float32)

    bf = ml_dtypes.bfloat16
    f8 = ml_dtypes.float8_e3m4

    x = X_batch[sel_idx.astype(np.int64)]                    # [N2, F]
    xt_host = np.ascontiguousarray(x.T.astype(bf))           # [F, N2]

    w1 = np.einsum("rb,bfe->rfe", comp1, bases1) / A0_SCALE  # [R, F, E]
    w2 = np.einsum("rb,bec->rec", comp2, bases2)             # [R, E, C]
    w2a_host = np.ascontiguousarray(
        w2.transpose(1, 0, 2).reshape(E, R * C))             # [E, R*C] f32

    a0q = (A0 * A0_SCALE).astype(f8)                         # [N1, R*N2] fp8
    a0T = a0q.T                                              # [R*N2, N1] view
    a1T = np.ascontiguousarray(A1.astype(bf).T)              # [R*N1, NOUT]

    b1_host = np.ascontiguousarray(bias1.reshape(E, 1))
    b2_host = np.ascontiguousarray(bias2.reshape(C, 1))

    chunk_off = [0]
    for w in CHUNKS:
        chunk_off.append(chunk_off[-1] + w)

    in_maps = []
    for c in range(NCORES):
        w1c_host = np.ascontiguousarray(
            np.concatenate([w1[RPC * c + i] for i in range(RPC)],
                           axis=1).astype(bf))
        # core's A0.T block [KPC, N1] pre-swizzled to DMA order:
        # for each chunk ch, [p, t, n'] with element (t*128+p, off_ch+n')
        blk = a0T[c * KPC:(c + 1) * KPC]                     # [KPC, N1]
        blk4 = blk.reshape(NKT, 128, N1).transpose(1, 0, 2)  # [p, t, n]
        parts = [np.ascontiguousarray(
                    blk4[:, :, chunk_off[ch]:chunk_off[ch + 1]]
                 ).reshape(128, NKT * w)
                 for ch, w in enumerate(CHUNKS)]
        a0_pack = np.ascontiguousarray(np.concatenate(parts, axis=1))
        # core c's 32 output rows: pack A1.T[:, 32c:32c+32] so each k-tile is
        # a [128, 32] slice living at a1t[:, t*32:(t+1)*32]
        a1_blk = a1T[:, NOPC * c:NOPC * (c + 1)]             # [R*N1, 32]
        a1_pack = np.ascontiguousarray(
            a1_blk.reshape(NKT, 128, C).transpose(1, 0, 2).reshape(128,
                                                                   NKT * C))
        in_maps.append({
            "xt": xt_host,
            "a0t": a0_pack,
            "a1t": a1_pack,
            "w1c": w1c_host,
            "w2a": w2a_host,
            "b1": b1_host,
            "b2": b2_host,
        })
    return in_maps


def kernel(X_batch, sel_idx, A0, A1, comp1, bases1, comp2, bases2,
           bias1, bias2):
    global last_results
    from concourse.bass_utils import run_bass_kernel_spmd

    in_maps = make_in_maps(X_batch, sel_idx, A0, A1, comp1, bases1,
                           comp2, bases2, bias1, bias2)
    nc = _get_module()
    res = run_bass_kernel_spmd(nc, in_maps, core_ids=list(range(NCORES)))
    last_results = res

    outT = np.concatenate([res.results[c]["outT"] for c in range(NCORES)],
                          axis=1)                            # [C, NOUT]
    return np.ascontiguousarray(outT.T)                      # [NOUT, C]


# revision 14
# speedup vs baseline: 1.7336x; 1.7336x over previous
"""Trainium2 Bass kernel: LADIES mini-batch ER-GCN (2-layer relational GCN).

Contract: kernel(**inputs) takes the FULL unsharded inputs (numpy, keyed as in
setup_inputs) and returns the FULL [256, 32] float32 output.

Strategy (8 NeuronCores, relation-sharded layer 1, output-row-sharded layer 2):
  - h1 = relu(A0 @ xw + b1) dominates: A0 is [1024, 131072] f32 = 512 MB.
    Core c owns relations {2c, 2c+1} = a contiguous 64 MB column block of A0.
    The block is transposed AND pre-swizzled into exact DMA order on the host,
    then streamed as fp8 e3m4 (scale 512 folded into w1; quantization error
    ~1.6e-2 rel-absmax, under the 2e-2 gate) -> 16.8 MB/core, contiguous
    per-partition runs, ~1.3 MB per dma_start on the sync HWDGE ring.
  - Each core computes xw[r] = x @ w1[r] for its 2 relations on-device (bf16),
    then accumulates h1.T = sum_k xw[k-tile].T @ A0T[k-tile] in PSUM, in 2
    asymmetric column chunks (640 then 384) so the first chunk's AllReduce
    hides under the second chunk's streaming and the exposed final AllReduce
    is small (48 KB bf16).
  - A tiny warmup AllReduce fires ~3 us into the kernel: it absorbs the ncfw
    first-collective warmup (~20-30 us) and synchronizes the 8 ranks so the
    real AllReduces start promptly when triggered.
  - Layer 2: after the AllReduce every core has full h1, so core c computes
    out.T[:, 32c:32c+32] (its 32 output rows) against a host-packed A1.T
    column block -- no second collective; the host concatenates the slices.
    The M=32 out-matmuls run 3-at-a-time in distinct PE column groups.
"""

import numpy as np
import ml_dtypes

# Problem dimensions (fixed by the problem spec).
R, NB = 16, 16
N2, N1, NOUT = 8192, 1024, 256
F, E, C = 128, 64, 32

NCORES = 8
RPC = R // NCORES            # relations per core = 2
KPC = RPC * N2               # layer-1 contraction rows per core = 16384
NKT = KPC // 128             # k-tiles per core = 128
NB2 = N2 // 128              # n2-blocks per relation = 64
CHUNKS = [640, 384]          # n1 column chunks (AllReduce pipelining)
KT_PER_DMA = 16              # k-tiles per A0 DMA (~1.3 MB fp8 transfers)
NOPC = NOUT // NCORES        # output rows per core = 32
XWB = 4                      # xw n2-blocks batched per PSUM bank/cast

A0_SCALE = 512.0             # power-of-2 scale folded into w1 (fp8 range fit)

_cache = {}
last_results = None          # BassKernelResults from the most recent run


def _subtiles(width):
    """Split a chunk width into PSUM-bank-sized (<=512 f32) sub-widths."""
    subs = []
    off = 0
    while off < width:
        w = min(512, width - off)
        subs.append((off, w))
        off += w
    return subs


def _build_module(repeats=1, use_collectives=True):
    import concourse.bacc as bacc
    import concourse.tile as tile
    import concourse.tile as tile_mod
    import concourse.mybir as mybir

    f32 = mybir.dt.float32
    bf16 = mybir.dt.bfloat16
    f8 = mybir.dt.float8e3

    nc = bacc.Bacc("TRN2", target_bir_lowering=False, debug=False,
                   num_devices=NCORES)

    xt = nc.dram_tensor("xt", [F, N2], bf16, kind="ExternalInput")
    a0t = nc.dram_tensor("a0t", [128, NKT * N1], f8, kind="ExternalInput")
    a1t = nc.dram_tensor("a1t", [128, NKT * C], bf16, kind="ExternalInput")
    w1c = nc.dram_tensor("w1c", [F, RPC * E], bf16, kind="ExternalInput")
    w2a = nc.dram_tensor("w2a", [E, R * C], f32, kind="ExternalInput")
    b1 = nc.dram_tensor("b1", [E, 1], f32, kind="ExternalInput")
    b2 = nc.dram_tensor("b2", [C, 1], f32, kind="ExternalInput")
    outT = nc.dram_tensor("outT", [C, NOPC], f32, kind="ExternalOutput")

    # host pre-swizzled: per chunk ch (column range), [p, t, n'] contiguous
    chunk_off = [0]
    for w in CHUNKS:
        chunk_off.append(chunk_off[-1] + w)
    rg = [list(range(NCORES))]

    with tile.TileContext(nc) as tc:
        with (
            tc.tile_pool(name="const", bufs=1) as constp,
            tc.tile_pool(name="xtp", bufs=1) as xtp,
            tc.tile_pool(name="xwp", bufs=1) as xwp,
            tc.tile_pool(name="a0p", bufs=8) as a0p,
            tc.tile_pool(name="a1p", bufs=1) as a1p,
            tc.tile_pool(name="h1p", bufs=2) as h1p,
            tc.tile_pool(name="h2p", bufs=8) as h2p,
            tc.tile_pool(name="warm", bufs=1) as warmp,
            tc.tile_pool(name="psxw", bufs=2, space="PSUM") as psxw,
            tc.tile_pool(name="psh1", bufs=1, space="PSUM") as psh1,
            tc.tile_pool(name="psh2", bufs=2, space="PSUM") as psh2,
            tc.tile_pool(name="psout", bufs=1, space="PSUM") as psoutp,
            tc.tile_pool(name="dram", bufs=1, space="DRAM") as dramp,
        ):
            # ---- input loads ----
            # xt first half leads the sync HWDGE ring (1 MB ahead of A0 --
            # costs the stream ~3.4 us but lets the xw matmuls start ~7 us
            # earlier, and PE paces the second chunk). Second half on the
            # scalar ring behind w1. a1 is queued on the sync ring between
            # the two A0 chunks (it is needed only in the post phase).
            xt_sb = [xtp.tile([F, N2 // 2], bf16, name=f"xt_sb{h}")
                     for h in range(2)]
            nc.sync.dma_start(xt_sb[0][:], xt[:, 0:N2 // 2])
            w1_sb = constp.tile([F, RPC * E], bf16, name="w1_sb")
            nc.scalar.dma_start(w1_sb[:], w1c[:])
            nc.scalar.dma_start(xt_sb[1][:], xt[:, N2 // 2:N2])
            w2_sb = constp.tile([E, R * C], f32, name="w2_sb")
            nc.scalar.dma_start(w2_sb[:], w2a[:])
            b1_sb = constp.tile([E, 1], f32, name="b1_sb")
            nc.scalar.dma_start(b1_sb[:], b1[:])
            b2_sb = constp.tile([C, 1], f32, name="b2_sb")
            nc.scalar.dma_start(b2_sb[:], b2[:])
            a1_sb = a1p.tile([128, NKT * C], bf16, name="a1_sb")

            for rep in range(repeats):
                # ---- xw[kt] = x[n2-block] @ w1[r_local] (kt = rl*64+nb) --
                # one matmul per n2-block computes BOTH relations (N=128).
                # XWB blocks share one PSUM bank so a single wide strided
                # cast drains them -- keeps PE matmuls back-to-back (HAM
                # warm) instead of serializing on per-block DVE casts.
                xw_sb = xwp.tile([128, NKT * E], bf16, name="xw_sb",
                                 tag="xw_sb")
                xw_v = xw_sb[:].rearrange("p (rl nb e) -> p nb rl e",
                                          rl=RPC, e=E)
                for g in range(NB2 // XWB):
                    ps = psxw.tile([128, XWB * RPC * E], f32, name="ps_xw",
                                   tag="ps_xw")
                    for q in range(XWB):
                        nb = g * XWB + q
                        xth = xt_sb[nb // (NB2 // 2)]
                        nbl = nb % (NB2 // 2)
                        nc.tensor.matmul(
                            ps[:, q * RPC * E:(q + 1) * RPC * E],
                            xth[:, nbl * 128:(nbl + 1) * 128],
                            w1_sb[:],
                            start=True, stop=True,
                        )
                    nc.vector.tensor_copy(
                        xw_v[:, g * XWB:(g + 1) * XWB],
                        ps[:].rearrange("p (nb rl e) -> p nb rl e",
                                        nb=XWB, e=E))

                # collective bounce buffers (one pair per chunk, bf16)
                cc_in = [dramp.tile([E, w], bf16, name=f"cc_in{rep}_{ch}")
                         for ch, w in enumerate(CHUNKS)]
                cc_out = [dramp.tile([E, w], bf16, name=f"cc_out{rep}_{ch}",
                                     addr_space="Shared")
                          for ch, w in enumerate(CHUNKS)]

                # ---- stream phase: all chunks' A0 matmuls, then their ARs --
                stream_mms = [[] for _ in CHUNKS]
                for ch, cw in enumerate(CHUNKS):
                    a0_view = a0t.ap()[:, chunk_off[ch] * NKT:
                                       chunk_off[ch] * NKT + NKT * cw]
                    a0_view = a0_view.rearrange("p (t n) -> p t n", n=cw)
                    subs = _subtiles(cw)
                    ps_h1 = [psh1.tile([E, sw], f32, name=f"ps_h1_{ch}_{si}",
                                       tag=f"ps_h1_{ch}_{si}")
                             for si, (soff, sw) in enumerate(subs)]
                    for g in range(NKT // KT_PER_DMA):
                        a0_sb = a0p.tile([128, KT_PER_DMA, cw], f8,
                                         name="a0_sb", tag="a0")
                        nc.sync.dma_start(
                            a0_sb[:],
                            a0_view[:, g * KT_PER_DMA:(g + 1) * KT_PER_DMA,
                                    :],
                        )
                        for i in range(KT_PER_DMA):
                            kt = g * KT_PER_DMA + i
                            for si, (soff, sw) in enumerate(subs):
                                mm = nc.tensor.matmul(
                                    ps_h1[si][:],
                                    xw_sb[:, kt * E:(kt + 1) * E],
                                    a0_sb[:, i, soff:soff + sw],
                                    start=(kt == 0), stop=(kt == NKT - 1),
                                )
                                stream_mms[ch].append(mm)
                    if ch == 0:
                        # a1 rides the sync ring between the two chunks
                        nc.sync.dma_start(a1_sb[:], a1t[:])
                    h1part = h1p.tile([E, cw], bf16, name=f"h1part{ch}",
                                      tag=f"h1part{ch}")
                    for si, (soff, sw) in enumerate(subs):
                        nc.vector.tensor_copy(h1part[:, soff:soff + sw],
                                              ps_h1[si][:])
                    nc.scalar.dma_start(cc_in[ch][:], h1part[:])
                    if use_collectives:
                        nc.gpsimd.collective_compute(
                            "AllReduce",
                            mybir.AluOpType.add,
                            replica_groups=rg,
                            ins=[cc_in[ch].opt()],
                            outs=[cc_out[ch].opt()],
                        )
                    else:  # single-core timing variant
                        nc.gpsimd.dma_start(cc_out[ch][:], cc_in[ch][:])

                # ---- post phase: per chunk, relu + layer 2 + out-MMs ----
                # out-MMs are M=32: run 3 concurrently in distinct PE column
                # groups (partition offsets 0/32/64 of one PSUM bank, via
                # auto tile_position), one accumulation group per r%3;
                # partials summed on DVE at the end.
                ps_out = psoutp.tile([96, NOPC], f32, name="ps_out",
                                     tag="ps_out")
                nblk_total = N1 // 128
                nper = [0, 0, 0]
                for r in range(R):
                    nper[r % 3] += nblk_total
                igrp = [0, 0, 0]
                for ch, cw in enumerate(CHUNKS):
                    nb1c = cw // 128
                    h1s = h1p.tile([E, cw], bf16, name=f"h1s{ch}",
                                   tag=f"h1s{ch}")
                    nc.scalar.dma_start(h1s[:], cc_out[ch][:])
                    h1r = h1p.tile([E, cw], f32, name=f"h1r{ch}",
                                   tag=f"h1r{ch}")
                    nc.scalar.activation(
                        h1r[:], h1s[:],
                        mybir.ActivationFunctionType.Relu,
                        bias=b1_sb[:],
                    )

                    # h2 for ALL 16 relations of one n1-block in a single
                    # N=512 matmul (out columns are (r, c) pairs, r-major,
                    # matching w2a/a1t layout). Anchor this chunk's PE work
                    # behind ~75% of the NEXT chunk's stream matmuls so the
                    # scheduler can't hoist it ahead and head-of-line-block
                    # PE on the AllReduce.
                    if ch + 1 < len(CHUNKS):
                        nxt = stream_mms[ch + 1]
                        anchor = nxt[(3 * len(nxt)) // 4].ins
                    else:
                        anchor = None
                    h2ts = {}
                    for b in range(nb1c):
                        ps2 = psh2.tile([128, R * C], f32, name="ps_h2",
                                        tag="ps_h2")
                        mm = nc.tensor.matmul(
                            ps2[:],
                            h1r[:, b * 128:(b + 1) * 128],
                            w2_sb[:],
                            start=True, stop=True,
                        )
                        if anchor is not None:
                            tile_mod.add_dep_helper(
                                mm.ins, anchor, sync=False,
                                reason="post-phase after next stream")
                        h2t = h2p.tile([128, R * C], bf16, name="h2t",
                                       tag="h2t")
                        nc.vector.tensor_copy(h2t[:], ps2[:])
                        h2ts[b] = h2t

                    for b in range(nb1c):
                        nb1 = chunk_off[ch] // 128 + b
                        for r in range(R):
                            t = r * (N1 // 128) + nb1
                            q = r % 3
                            nc.tensor.matmul(
                                ps_out[32 * q:32 * (q + 1), :],
                                h2ts[b][:, r * C:(r + 1) * C],
                                a1_sb[:, t * C:(t + 1) * C],
                                start=(igrp[q] == 0),
                                stop=(igrp[q] == nper[q] - 1),
                                skip_group_check=True,
                            )
                            igrp[q] += 1

                # ---- sum the 3 col-group partials + bias2, store out.T ----
                t01 = constp.tile([C, NOPC], f32, name="t01", tag="t01")
                out_sb = constp.tile([C, NOPC], f32, name="out_sb",
                                     tag="out_sb")
                nc.vector.tensor_copy(t01[:], ps_out[0:32, :])
                nc.vector.tensor_add(t01[:], ps_out[32:64, :], t01[:])
                nc.vector.tensor_add(t01[:], ps_out[64:96, :], t01[:])
                nc.vector.tensor_scalar_add(out_sb[:], t01[:], b2_sb[:])
                nc.scalar.dma_start(outT[:], out_sb[:])


    nc.compile()
    return nc


def _get_module():
    if "nc" not in _cache:
        _cache["nc"] = _build_module()
    return _cache["nc"]


def make_in_maps(X_batch, sel_idx, A0, A1, comp1, bases1, comp2, bases2,
                 bias1, bias2):
    """Host-side sharding / layout prep -> per-core input maps."""
    X_batch = np.asarray(X_batch, dtype=np.float32)
    sel_idx = np.asarray(sel_idx)
    A0 = np.asarray(A0, dtype=np.float32)
    A1 = np.asarray(A1, dtype=np.float32)
    comp1 = np.asarray(comp1, dtype=np.float32)
    bases1 = np.asarray(bases1, dtype=np.float32)
    comp2 = np.asarray(comp2, dtype=np.float32)
    bases2 = np.asarray(bases2, dtype=np.float32)
    bias1 = np.asarray(bias1, dtype=np.float32)
    bias2 = np.asarray(bias2, dtype=np.float32)

    bf = ml_dtypes.bfloat16
    f8 = ml_dtypes.float8_e3m4

    x = X_batch[sel_idx.astype(np.int64)]                    # [N2, F]
    xt_host = np.ascontiguousarray(x.T.astype(bf))           # [F, N2]

    w1 = np.einsum("rb,bfe->rfe", comp1, bases1) / A0_SCALE  # [R, F, E]
    w2 = np.einsum("rb,bec->rec", comp2, bases2)             # [R, E, C]
    w2a_host = np.ascontiguousarray(
        w2.transpose(1, 0, 2).reshape(E, R * C))             # [E, R*C] f32

    a0q = (A0 * A0_SCALE).astype(f8)                         # [N1, R*N2] fp8
    a0T = a0q.T                                              # [R*N2, N1] view
    a1T = np.ascontiguousarray(A1.astype(bf).T)              # [R*N1, NOUT]

    b1_host = np.ascontiguousarray(bias1.reshape(E, 1))
    b2_host = np.ascontiguousarray(bias2.reshape(C, 1))

    chunk_off = [0]
    for w in CHUNKS:
        chunk_off.append(chunk_off[-1] + w)

    in_maps = []
    for c in range(NCORES):
        w1c_host = np.ascontiguousarray(
            np.concatenate([w1[RPC * c + i] for i in range(RPC)],
                           axis=1).astype(bf))
        # core's A0.T block [KPC, N1] pre-swizzled to DMA order:
        # for each chunk ch, [p, t, n'] with element (t*128+p, off_ch+n')
        blk = a0T[c * KPC:(c + 1) * KPC]                     # [KPC, N1]
        blk4 = blk.reshape(NKT, 128, N1).transpose(1, 0, 2)  # [p, t, n]
        parts = [np.ascontiguousarray(
                    blk4[:, :, chunk_off[ch]:chunk_off[ch + 1]]
                 ).reshape(128, NKT * w)
                 for ch, w in enumerate(CHUNKS)]
        a0_pack = np.ascontiguousarray(np.concatenate(parts, axis=1))
        # core c's 32 output rows: pack A1.T[:, 32c:32c+32] so each k-tile is
        # a [128, 32] slice living at a1t[:, t*32:(t+1)*32]
        a1_blk = a1T[:, NOPC * c:NOPC * (c + 1)]             # [R*N1, 32]
        a1_pack = np.ascontiguousarray(
            a1_blk.reshape(NKT, 128, C).transpose(1, 0, 2).reshape(128,
                                                                   NKT * C))
        in_maps.append({
            "xt": xt_host,
            "a0t": a0_pack,
            "a1t": a1_pack,
            "w1c": w1c_host,
            "w2a": w2a_host,
            "b1": b1_host,
            "b2": b2_host,
        })
    return in_maps


def kernel(X_batch, sel_idx, A0, A1, comp1, bases1, comp2, bases2,
           bias1, bias2):
    global last_results
    from concourse.bass_utils import run_bass_kernel_spmd

    in_maps = make_in_maps(X_batch, sel_idx, A0, A1, comp1, bases1,
                           comp2, bases2, bias1, bias2)
    nc = _get_module()
    res = run_bass_kernel_spmd(nc, in_maps, core_ids=list(range(NCORES)))
    last_results = res

    outT = np.concatenate([res.results[c]["outT"] for c in range(NCORES)],
                          axis=1)                            # [C, NOUT]
    return np.ascontiguousarray(outT.T)                      # [NOUT, C]
